# revision 5
# baseline (speedup 1.0000x reference)
"""Bass/Trainium2 kernel for nn_BmmEnsemble (ensemble-of-MLPs atomic energy sum).

Sharding: 8 cores; core c owns species c//2, half c%2 (12500/2 = 6250 atoms).
Each core runs a 3-layer MLP (1008->256->192->160, CELU) for its species'
8 ensemble members on its 6250 atoms, producing per-feature sums of the
layer-3 activations. Layer 4 ([160]->[1]), the ensemble mean, the CELU
constant shifts and the final atom sum are all linear, so they are folded
into host-side fp64 math on the tiny reduced vectors.

Device layout: activations are [features(partitions) x atoms(free)], weights
are natural [din(partitions) x dout(free)] so NO on-device transposes are
needed; the host supplies aev pre-gathered and transposed per core.

CELU algebra (alpha=0.1):
  celu(x) = relu(x) + alpha*min(exp(x/alpha), 1) - alpha
          = relu(x) + min(alpha*exp(x/alpha), alpha) - alpha
  with x = z + b.  Device computes g = relu(z+b) + min(exp(10*z + (10*b+ln a)), a)
  (one ACT Exp + one DVE relu + one DVE scalar_tensor_tensor per tile).
  The "- alpha" shifts fold into the next layer's bias:
  b_adj = b - alpha * colsum(W).  Layer-3 feature sums are accumulated via
  DVE accum_out into per-(member, chunk) columns and finished on the host.
"""

import numpy as np

import concourse.bacc as bacc
import concourse.tile as tile
import concourse.mybir as mybir
from concourse.bass_utils import run_bass_kernel_spmd

F32 = mybir.dt.float32
F32R = mybir.dt.float32r
AF = mybir.ActivationFunctionType
ALU = mybir.AluOpType

S = 4
E = 8
N = 50000
AEV = 1008
ALPHA = 0.1
NCORES = 8
NA = N // S // 2          # atoms per core: 6250
CH = 512                  # atom chunk (matmul free dim)
NCH = (NA + CH - 1) // CH  # 13 chunks (12 x 512 + 106)
D1, D2, D3 = 256, 192, 160
KC1 = [(kc * 128, min(128, AEV - kc * 128)) for kc in range((AEV + 127) // 128)]  # 8 chunks
L2K = [(0, 128), (128, 128)]          # K chunks of 256
L2M = [(0, 128), (128, 64)]           # M chunks of 192
L3K = [(0, 128), (128, 64)]           # K chunks of 192
L3M = [(0, 128), (128, 32)]           # M chunks of 160
NCOL = E * NCH                        # accumulator columns (one per member x chunk)

USE_F32R = True


def _build():
    nc = bacc.Bacc("TRN2", target_bir_lowering=False, debug=False, num_devices=NCORES)

    MMDT = F32R if USE_F32R else F32
    xT = nc.dram_tensor("xT", [AEV, NA], MMDT, kind="ExternalInput")
    w1 = nc.dram_tensor("w1", [E, AEV, D1], MMDT, kind="ExternalInput")
    w2 = nc.dram_tensor("w2", [E, 256, D2], MMDT, kind="ExternalInput")
    w3 = nc.dram_tensor("w3", [E, 192, D3], MMDT, kind="ExternalInput")
    b1 = nc.dram_tensor("b1", [E, 128, 4], F32, kind="ExternalInput")
    b2 = nc.dram_tensor("b2", [E, 128, 4], F32, kind="ExternalInput")
    b3 = nc.dram_tensor("b3", [E, 128, 4], F32, kind="ExternalInput")
    acc1 = nc.dram_tensor("acc1", [128, NCOL], F32, kind="ExternalOutput")
    acc2 = nc.dram_tensor("acc2", [32, NCOL], F32, kind="ExternalOutput")

    with tile.TileContext(nc) as tc:
        with (
            tc.tile_pool(name="wp", bufs=1) as wp,
            tc.tile_pool(name="xp", bufs=2) as xp,
            tc.tile_pool(name="gp", bufs=4) as gp,
            tc.tile_pool(name="sp", bufs=3) as sp,
            tc.tile_pool(name="ps", bufs=8, space="PSUM") as ps,
        ):
            # ---- resident weights / biases / accumulators ----
            w1t = {}
            for e in range(E):
                for kc, (k0, kw) in enumerate(KC1):
                    t = wp.tile([kw, D1], MMDT, tag=f"w1_{e}_{kc}")
                    nc.sync.dma_start(t[:], w1[e, k0:k0 + kw, :])
                    w1t[e, kc] = t
            w2t = {}
            for e in range(E):
                for kc, (k0, kw) in enumerate(L2K):
                    t = wp.tile([kw, D2], MMDT, tag=f"w2_{e}_{kc}")
                    nc.sync.dma_start(t[:], w2[e, k0:k0 + kw, :])
                    w2t[e, kc] = t
            w3t = {}
            for e in range(E):
                for kc, (k0, kw) in enumerate(L3K):
                    t = wp.tile([kw, D3], MMDT, tag=f"w3_{e}_{kc}")
                    nc.sync.dma_start(t[:], w3[e, k0:k0 + kw, :])
                    w3t[e, kc] = t
            b1t, b2t, b3t = {}, {}, {}
            for e in range(E):
                for name, dram, store in (("b1", b1, b1t), ("b2", b2, b2t), ("b3", b3, b3t)):
                    t = wp.tile([128, 4], F32, tag=f"{name}_{e}")
                    nc.sync.dma_start(t[:], dram[e])
                    store[e] = t
            acc1t = wp.tile([128, NCOL], F32, tag="acc1")
            acc2t = wp.tile([32, NCOL], F32, tag="acc2")

            # ---- main loop ----
            for ci in range(NCH):
                off = ci * CH
                na = min(CH, NA - off)
                xt = []
                for kc, (k0, kw) in enumerate(KC1):
                    t = xp.tile([kw, CH], MMDT, tag=f"x_{kc}")
                    nc.sync.dma_start(t[:, :na], xT[k0:k0 + kw, off:off + na])
                    xt.append(t)
                for e in range(E):
                    # ----- layer 1: [1008] -> [256] -----
                    g1 = []
                    for m in range(2):
                        z = ps.tile([128, CH], F32, tag="z")
                        zv = z[:, :na]
                        for kc, (k0, kw) in enumerate(KC1):
                            nc.tensor.matmul(
                                zv,
                                (w1t[e, kc][:, m * 128:(m + 1) * 128]),
                                (xt[kc][:kw, :na]),
                                start=(kc == 0),
                                stop=(kc == len(KC1) - 1),
                            )
                        u = sp.tile([128, CH], F32, tag="u")
                        nc.scalar.activation(u[:, :na], zv, AF.Exp,
                                             bias=b1t[e][:, 2 * m + 1:2 * m + 2], scale=10.0)
                        r = sp.tile([128, CH], F32, tag="r")
                        nc.vector.tensor_scalar(r[:, :na], zv, b1t[e][:, 2 * m:2 * m + 1], 0.0,
                                                op0=ALU.add, op1=ALU.max)
                        g = gp.tile([128, CH], MMDT, tag="g1")
                        nc.vector.scalar_tensor_tensor(g[:, :na], u[:, :na], ALPHA, r[:, :na],
                                                       op0=ALU.min, op1=ALU.add)
                        g1.append(g)
                    # ----- layer 2: [256] -> [192] -----
                    g2 = []
                    for mi, (m0, mw) in enumerate(L2M):
                        z = ps.tile([128, CH], F32, tag="z")
                        zv = z[:mw, :na]
                        for kc, (k0, kw) in enumerate(L2K):
                            nc.tensor.matmul(
                                zv,
                                (w2t[e, kc][:, m0:m0 + mw]),
                                (g1[kc][:, :na]),
                                start=(kc == 0),
                                stop=(kc == len(L2K) - 1),
                            )
                        u = sp.tile([128, CH], F32, tag="u")
                        nc.scalar.activation(u[:mw, :na], zv, AF.Exp,
                                             bias=b2t[e][:mw, 2 * mi + 1:2 * mi + 2], scale=10.0)
                        r = sp.tile([128, CH], F32, tag="r")
                        nc.vector.tensor_scalar(r[:mw, :na], zv, b2t[e][:mw, 2 * mi:2 * mi + 1], 0.0,
                                                op0=ALU.add, op1=ALU.max)
                        g = gp.tile([128, CH], MMDT, tag="g2")
                        nc.vector.scalar_tensor_tensor(g[:mw, :na], u[:mw, :na], ALPHA, r[:mw, :na],
                                                       op0=ALU.min, op1=ALU.add)
                        g2.append(g)
                    # ----- layer 3: [192] -> [160], reduced over atoms -----
                    for mi, (m0, mw) in enumerate(L3M):
                        z = ps.tile([128, CH], F32, tag="z")
                        zv = z[:mw, :na]
                        for kc, (k0, kw) in enumerate(L3K):
                            nc.tensor.matmul(
                                zv,
                                (w3t[e, kc][:, m0:m0 + mw]),
                                (g2[kc][:kw, :na]),
                                start=(kc == 0),
                                stop=(kc == len(L3K) - 1),
                            )
                        accT = acc1t if mi == 0 else acc2t
                        col = e * NCH + ci
                        u = sp.tile([128, CH], F32, tag="u")
                        nc.scalar.activation(u[:mw, :na], zv, AF.Exp,
                                             bias=b3t[e][:mw, 2 * mi + 1:2 * mi + 2], scale=10.0)
                        r = sp.tile([128, CH], F32, tag="r")
                        nc.vector.tensor_scalar(r[:mw, :na], zv, b3t[e][:mw, 2 * mi:2 * mi + 1], 0.0,
                                                op0=ALU.add, op1=ALU.max)
                        s2 = sp.tile([128, CH], F32, tag="s2")
                        nc.vector.scalar_tensor_tensor(s2[:mw, :na], u[:mw, :na], ALPHA, r[:mw, :na],
                                                       op0=ALU.min, op1=ALU.add,
                                                       accum_out=accT[:, col:col + 1])
            nc.sync.dma_start(acc1[:], acc1t[:])
            nc.sync.dma_start(acc2[:], acc2t[:])
    nc.compile()
    return nc


_NC = None


def _get_nc():
    global _NC
    if _NC is None:
        _NC = _build()
    return _NC


def _prep_inputs(inputs):
    aev = np.asarray(inputs["aev"], dtype=np.float32).reshape(N, AEV)
    idx = np.asarray(inputs["idx"])
    Ws = [np.asarray(inputs[f"W{i}"], dtype=np.float32) for i in (1, 2, 3, 4)]
    bs = [np.asarray(inputs[f"b{i}"], dtype=np.float32) for i in (1, 2, 3, 4)]

    ln_a = float(np.log(ALPHA))
    in_maps = []
    sels = []
    for c in range(NCORES):
        s, h = c // 2, c % 2
        sel = np.asarray(idx[s, h * NA:(h + 1) * NA])
        sels.append(sel)
        xTc = np.ascontiguousarray(aev[sel].T)  # [1008, 6250]

        w1c = np.ascontiguousarray(Ws[0][s])    # [8, 1008, 256]
        w2c = np.ascontiguousarray(Ws[1][s])    # [8, 256, 192]
        w3c = np.ascontiguousarray(Ws[2][s])    # [8, 192, 160]

        b1v = bs[0][s][:, 0, :].astype(np.float64)                       # [8, 256]
        b2v = bs[1][s][:, 0, :].astype(np.float64) \
            - ALPHA * Ws[1][s].astype(np.float64).sum(axis=1)            # [8, 192]
        b3v = bs[2][s][:, 0, :].astype(np.float64) \
            - ALPHA * Ws[2][s].astype(np.float64).sum(axis=1)            # [8, 160]

        def pack(bv, chunks):
            out = np.zeros((E, 128, 4), dtype=np.float32)
            for mi, (m0, mw) in enumerate(chunks):
                out[:, :mw, 2 * mi] = bv[:, m0:m0 + mw]
                out[:, :mw, 2 * mi + 1] = 10.0 * bv[:, m0:m0 + mw] + ln_a
            return out

        in_maps.append({
            "xT": xTc,
            "w1": w1c, "w2": w2c, "w3": w3c,
            "b1": pack(b1v, [(0, 128), (128, 128)]),
            "b2": pack(b2v, L2M),
            "b3": pack(b3v, L3M),
        })
    return in_maps, sels, Ws, bs


def _finish(results, Ws, bs):
    W4 = Ws[3].astype(np.float64)  # [S, E, 160, 1]
    b4 = bs[3].astype(np.float64)  # [S, E, 1, 1]
    total = 0.0
    for c in range(NCORES):
        s = c // 2
        a1 = results[c]["acc1"].astype(np.float64)  # [128, NCOL]
        a2 = results[c]["acc2"].astype(np.float64)  # [32, NCOL]
        for e in range(E):
            cols = [e * NCH + ci for ci in range(NCH)]
            g3sum = np.concatenate([
                a1[:, cols].sum(axis=1),
                a2[:, cols].sum(axis=1),
            ])  # [160]
            h3sum = g3sum - ALPHA * NA
            total += (h3sum @ W4[s, e, :, 0] + NA * b4[s, e, 0, 0]) / E
    return np.array([total], dtype=np.float32)


def _run(inputs, **spmd_kwargs):
    in_maps, sels, Ws, bs = _prep_inputs(inputs)
    nc = _get_nc()
    res = run_bass_kernel_spmd(nc, in_maps, list(range(NCORES)), **spmd_kwargs)
    return _finish(res.results, Ws, bs), res


def kernel(**inputs) -> np.ndarray:
    out, _ = _run(inputs)
    return out


# revision 7
# speedup vs baseline: 2.0799x; 2.0799x over previous
"""Bass/Trainium2 kernel for nn_BmmEnsemble (ensemble-of-MLPs atomic energy sum).

Sharding: 8 cores; core c owns species c//2, half c%2 (12500/2 = 6250 atoms).
Each core runs a 3-layer MLP (1008->256->192->160, CELU) for its species'
8 ensemble members on its 6250 atoms, producing per-feature sums of the
layer-3 activations. Layer 4 ([160]->[1]), the ensemble mean, the CELU
constant shifts and the final atom sum are all linear, so they are folded
into host-side fp64 math on the tiny reduced vectors.

Device layout: activations are [features(partitions) x atoms(free)], weights
are natural [din(partitions) x dout(free)] so NO on-device transposes are
needed; the host supplies aev pre-gathered and transposed per core (with a
ones row appended so the layer-1 bias rides the matmul).

CELU algebra (alpha=0.1):
  g(x) := celu(x) + alpha = max(x + alpha, min(alpha*e^(x/alpha), alpha))
  (exact: for x>=0 both relu branch and saturated exp-min give x+alpha;
   for x<0 the exp branch wins since alpha*e^(x/alpha) >= x+alpha).
  The "+alpha" shift folds into the next layer's bias:
  b_adj = b - alpha * colsum(W).
  Layer 1: z' = z + b1 + alpha from the matmul ones-row; then
    u' = ACT Exp(10*z' + (ln a - 1)) = a*e^(x/alpha);  g = DVE stt (u' min a) max z'.
  Layer 2: u' = ACT Exp(10*z + 10*b_adj + ln a);  r = DVE (z add b_adj) max 0;
    g = DVE stt (u' min a) add r   [g = relu + min form, same value].
  Layer 3 (only per-feature atom sums needed):
    Sum g3 = Sum relu(x3) + Sum min(u3', a):
    ACT Relu(z3 + b_adj) with accum_out  +  DVE (u3' min a) add 0 with accum_out.
  Host: h3 = g3 - alpha, then layer 4 / ensemble mean / b4 terms in fp64.

Emission is software-pipelined per slot t: PE runs [L1(t), L2(t-1), L3(t-2)]
so the PE queue never waits on the elementwise chain of the same iteration.
"""

import numpy as np

import concourse.bacc as bacc
import concourse.tile as tile
import concourse.mybir as mybir
from concourse.bass_utils import run_bass_kernel_spmd

F32 = mybir.dt.float32
F32R = mybir.dt.float32r
AF = mybir.ActivationFunctionType
ALU = mybir.AluOpType

S = 4
E = 8
N = 50000
AEV = 1008
ALPHA = 0.1
LN_A = float(np.log(ALPHA))
NCORES = 8
NA = N // S // 2           # atoms per core: 6250
CH = 512                   # atom chunk (matmul free dim)
NCH = (NA + CH - 1) // CH  # 13 chunks (12 x 512 + 106)
D1, D2, D3 = 256, 192, 160
K1 = AEV + 1               # 1009: aev + ones row for bias
KC1 = [(kc * 128, min(128, K1 - kc * 128)) for kc in range((K1 + 127) // 128)]  # 8 chunks
L2K = [(0, 128), (128, 128)]          # K chunks of 256
L2M = [(0, 128), (128, 64)]           # M chunks of 192
L3K = [(0, 128), (128, 64)]           # K chunks of 192
L3M = [(0, 128), (128, 32)]           # M chunks of 160
NCOL = E * NCH * 2                    # accum columns: (e, chunk) x {relu, minexp}
NSLOT = E * NCH                       # 104 pipeline slots

USE_F32R = True


def _build():
    nc = bacc.Bacc("TRN2", target_bir_lowering=False, debug=False, num_devices=NCORES)

    MMDT = F32R if USE_F32R else F32
    xT = nc.dram_tensor("xT", [K1, NA], MMDT, kind="ExternalInput")
    w1 = nc.dram_tensor("w1", [E, K1, D1], MMDT, kind="ExternalInput")
    w2 = nc.dram_tensor("w2", [E, 256, D2], MMDT, kind="ExternalInput")
    w3 = nc.dram_tensor("w3", [E, 192, D3], MMDT, kind="ExternalInput")
    b2 = nc.dram_tensor("b2", [E, 128, 4], F32, kind="ExternalInput")
    b3 = nc.dram_tensor("b3", [E, 128, 4], F32, kind="ExternalInput")
    acc1 = nc.dram_tensor("acc1", [128, NCOL], F32, kind="ExternalOutput")
    acc2 = nc.dram_tensor("acc2", [32, NCOL], F32, kind="ExternalOutput")

    with tile.TileContext(nc) as tc:
        with (
            tc.tile_pool(name="wp", bufs=1) as wp,
            tc.tile_pool(name="xp", bufs=2) as xp,
            tc.tile_pool(name="gp", bufs=4) as gp,
            tc.tile_pool(name="sp", bufs=4) as sp,
            tc.tile_pool(name="ps", bufs=8, space="PSUM") as ps,
        ):
            # ---- x prefetch helper ----
            xtiles = {}   # ci -> list of tiles

            def emit_x_dma(ci):
                if ci >= NCH or ci in xtiles:
                    return
                off = ci * CH
                na = min(CH, NA - off)
                lst = []
                for kc, (k0, kw) in enumerate(KC1):
                    t = xp.tile([kw, CH], MMDT, tag=f"x_{kc}")
                    nc.sync.dma_start(t[:, :na], xT[k0:k0 + kw, off:off + na])
                    lst.append(t)
                xtiles[ci] = lst

            emit_x_dma(0)
            emit_x_dma(1)

            # ---- resident weights / biases (e-major so e=0 lands first) ----
            w1t, w2t, w3t, b2t, b3t = {}, {}, {}, {}, {}
            for e in range(E):
                for kc, (k0, kw) in enumerate(KC1):
                    t = wp.tile([kw, D1], MMDT, tag=f"w1_{e}_{kc}")
                    nc.sync.dma_start(t[:], w1[e, k0:k0 + kw, :])
                    w1t[e, kc] = t
                for kc, (k0, kw) in enumerate(L2K):
                    t = wp.tile([kw, D2], MMDT, tag=f"w2_{e}_{kc}")
                    nc.sync.dma_start(t[:], w2[e, k0:k0 + kw, :])
                    w2t[e, kc] = t
                for kc, (k0, kw) in enumerate(L3K):
                    t = wp.tile([kw, D3], MMDT, tag=f"w3_{e}_{kc}")
                    nc.sync.dma_start(t[:], w3[e, k0:k0 + kw, :])
                    w3t[e, kc] = t
                t = wp.tile([128, 4], F32, tag=f"b2_{e}")
                nc.sync.dma_start(t[:], b2[e])
                b2t[e] = t
                t = wp.tile([128, 4], F32, tag=f"b3_{e}")
                nc.sync.dma_start(t[:], b3[e])
                b3t[e] = t
            b1c = wp.tile([128, 1], F32, tag="b1c")
            nc.vector.memset(b1c[:], LN_A - 1.0)
            acc1t = wp.tile([128, NCOL], F32, tag="acc1")
            acc2t = wp.tile([32, NCOL], F32, tag="acc2")

            # ---- pipeline state ----
            state = {}  # it -> dict with z1, g1, z2, g2, z3 lists

            def slot_info(it):
                ci, e = divmod(it, E)
                na = min(CH, NA - ci * CH)
                return ci, e, na

            def l1_mm(it):
                ci, e, na = slot_info(it)
                if it % E == 0:
                    emit_x_dma(ci + 1)
                st = state.setdefault(it, {})
                st["z1"] = []
                for m in range(2):
                    z = ps.tile([128, CH], F32, tag="z")
                    zv = z[:, :na]
                    for kc, (k0, kw) in enumerate(KC1):
                        nc.tensor.matmul(
                            zv,
                            w1t[e, kc][:, m * 128:(m + 1) * 128],
                            xtiles[ci][kc][:kw, :na],
                            start=(kc == 0),
                            stop=(kc == len(KC1) - 1),
                        )
                    st["z1"].append(z)

            def l1_ew(it):
                ci, e, na = slot_info(it)
                st = state[it]
                st["g1"] = []
                for m in range(2):
                    zv = st["z1"][m][:, :na]
                    u = sp.tile([128, CH], F32, tag="u1")
                    nc.scalar.activation(u[:, :na], zv, AF.Exp, bias=b1c[:, 0:1], scale=10.0)
                    g = gp.tile([128, CH], MMDT, tag="g1")
                    nc.vector.scalar_tensor_tensor(g[:, :na], u[:, :na], ALPHA, zv,
                                                   op0=ALU.min, op1=ALU.max)
                    st["g1"].append(g)

            def l2_mm(it):
                ci, e, na = slot_info(it)
                st = state[it]
                st["z2"] = []
                for mi, (m0, mw) in enumerate(L2M):
                    z = ps.tile([128, CH], F32, tag="z")
                    zv = z[:mw, :na]
                    for kc, (k0, kw) in enumerate(L2K):
                        nc.tensor.matmul(
                            zv,
                            w2t[e, kc][:, m0:m0 + mw],
                            st["g1"][kc][:, :na],
                            start=(kc == 0),
                            stop=(kc == len(L2K) - 1),
                        )
                    st["z2"].append(z)

            def l2_ew(it):
                ci, e, na = slot_info(it)
                st = state[it]
                st["g2"] = []
                for mi, (m0, mw) in enumerate(L2M):
                    zv = st["z2"][mi][:mw, :na]
                    u = sp.tile([128, CH], F32, tag="u2")
                    nc.scalar.activation(u[:mw, :na], zv, AF.Exp,
                                         bias=b2t[e][:mw, 2 * mi + 1:2 * mi + 2], scale=10.0)
                    r = sp.tile([128, CH], F32, tag="r2")
                    nc.vector.tensor_scalar(r[:mw, :na], zv, b2t[e][:mw, 2 * mi:2 * mi + 1], 0.0,
                                            op0=ALU.add, op1=ALU.max)
                    g = gp.tile([128, CH], MMDT, tag="g2")
                    nc.vector.scalar_tensor_tensor(g[:mw, :na], u[:mw, :na], ALPHA, r[:mw, :na],
                                                   op0=ALU.min, op1=ALU.add)
                    st["g2"].append(g)

            def l3_mm(it):
                ci, e, na = slot_info(it)
                st = state[it]
                st["z3"] = []
                for mi, (m0, mw) in enumerate(L3M):
                    z = ps.tile([128, CH], F32, tag="z")
                    zv = z[:mw, :na]
                    for kc, (k0, kw) in enumerate(L3K):
                        nc.tensor.matmul(
                            zv,
                            w3t[e, kc][:, m0:m0 + mw],
                            st["g2"][kc][:kw, :na],
                            start=(kc == 0),
                            stop=(kc == len(L3K) - 1),
                        )
                    st["z3"].append(z)

            def l3_ew(it):
                ci, e, na = slot_info(it)
                st = state[it]
                for mi, (m0, mw) in enumerate(L3M):
                    zv = st["z3"][mi][:mw, :na]
                    accT = acc1t if mi == 0 else acc2t
                    col = (e * NCH + ci) * 2
                    # Sum relu(z+b) on ACT (accum_out = free-dim sum)
                    r = sp.tile([128, CH], F32, tag="r3")
                    nc.scalar.activation(r[:mw, :na], zv, AF.Relu,
                                         bias=b3t[e][:mw, 2 * mi:2 * mi + 1], scale=1.0,
                                         accum_out=accT[:, col:col + 1])
                    # Sum min(u', alpha) on DVE (op1=add doubles as the reduce op)
                    u = sp.tile([128, CH], F32, tag="u3")
                    nc.scalar.activation(u[:mw, :na], zv, AF.Exp,
                                         bias=b3t[e][:mw, 2 * mi + 1:2 * mi + 2], scale=10.0)
                    s2 = sp.tile([128, CH], F32, tag="s3")
                    nc.vector.tensor_scalar(s2[:mw, :na], u[:mw, :na], ALPHA, 0.0,
                                            op0=ALU.min, op1=ALU.add,
                                            accum_out=accT[:, col + 1:col + 2])
                # free pipeline state
                del state[it]

            # ---- software-pipelined main loop ----
            for t in range(NSLOT + 2):
                if t < NSLOT:
                    l1_mm(t)
                    l1_ew(t)
                if 1 <= t <= NSLOT:
                    l2_mm(t - 1)
                    l2_ew(t - 1)
                if t >= 2:
                    l3_mm(t - 2)
                    l3_ew(t - 2)

            nc.sync.dma_start(acc1[:], acc1t[:])
            nc.sync.dma_start(acc2[:], acc2t[:])
    nc.compile()
    return nc


_NC = None


def _get_nc():
    global _NC
    if _NC is None:
        _NC = _build()
    return _NC


def _prep_inputs(inputs):
    aev = np.asarray(inputs["aev"], dtype=np.float32).reshape(N, AEV)
    idx = np.asarray(inputs["idx"])
    Ws = [np.asarray(inputs[f"W{i}"], dtype=np.float32) for i in (1, 2, 3, 4)]
    bs = [np.asarray(inputs[f"b{i}"], dtype=np.float32) for i in (1, 2, 3, 4)]

    in_maps = []
    for c in range(NCORES):
        s, h = c // 2, c % 2
        sel = np.asarray(idx[s, h * NA:(h + 1) * NA])
        xTc = np.empty((K1, NA), dtype=np.float32)
        xTc[:AEV] = aev[sel].T
        xTc[AEV] = 1.0

        # layer-1 weights with bias+alpha ones-row
        w1c = np.empty((E, K1, D1), dtype=np.float32)
        w1c[:, :AEV, :] = Ws[0][s]
        w1c[:, AEV, :] = bs[0][s][:, 0, :] + ALPHA
        w2c = np.ascontiguousarray(Ws[1][s])    # [8, 256, 192]
        w3c = np.ascontiguousarray(Ws[2][s])    # [8, 192, 160]

        b2v = bs[1][s][:, 0, :].astype(np.float64) \
            - ALPHA * Ws[1][s].astype(np.float64).sum(axis=1)            # [8, 192]
        b3v = bs[2][s][:, 0, :].astype(np.float64) \
            - ALPHA * Ws[2][s].astype(np.float64).sum(axis=1)            # [8, 160]

        def pack(bv, chunks):
            out = np.zeros((E, 128, 4), dtype=np.float32)
            for mi, (m0, mw) in enumerate(chunks):
                out[:, :mw, 2 * mi] = bv[:, m0:m0 + mw]
                out[:, :mw, 2 * mi + 1] = 10.0 * bv[:, m0:m0 + mw] + LN_A
            return out

        in_maps.append({
            "xT": xTc,
            "w1": w1c, "w2": w2c, "w3": w3c,
            "b2": pack(b2v, L2M),
            "b3": pack(b3v, L3M),
        })
    return in_maps, Ws, bs


def _finish(results, Ws, bs):
    W4 = Ws[3].astype(np.float64)  # [S, E, 160, 1]
    b4 = bs[3].astype(np.float64)  # [S, E, 1, 1]
    total = 0.0
    for c in range(NCORES):
        s = c // 2
        a1 = results[c]["acc1"].astype(np.float64)  # [128, NCOL]
        a2 = results[c]["acc2"].astype(np.float64)  # [32, NCOL]
        for e in range(E):
            cols = [(e * NCH + ci) * 2 for ci in range(NCH)]
            colsm = [cc + 1 for cc in cols]
            g3sum = np.concatenate([
                a1[:, cols].sum(axis=1) + a1[:, colsm].sum(axis=1),
                a2[:, cols].sum(axis=1) + a2[:, colsm].sum(axis=1),
            ])  # [160]
            h3sum = g3sum - ALPHA * NA
            total += (h3sum @ W4[s, e, :, 0] + NA * b4[s, e, 0, 0]) / E
    return np.array([total], dtype=np.float32)


def _run(inputs, **spmd_kwargs):
    in_maps, Ws, bs = _prep_inputs(inputs)
    nc = _get_nc()
    res = run_bass_kernel_spmd(nc, in_maps, list(range(NCORES)), **spmd_kwargs)
    return _finish(res.results, Ws, bs), res


def kernel(**inputs) -> np.ndarray:
    out, _ = _run(inputs)
    return out
